# revision 1
# baseline (speedup 1.0000x reference)
"""IsoGMM loss kernel for 8 Trainium2 NeuronCores.

loss = mean_{n,k} r[n,k] * ||X[n] - mus[k]||^2

Decomposition (the entire loss folds into ONE accumulated PE matmul per core):
  sum_{n,k} r*d2 = T1 + T2 - 2*T3
    T1 = sum_n xsq_n * R_n        (xsq_n = ||X[n]||^2, R_n = sum_k r[n,k])
    T2 = sum_k musq_k * C_k       (C_k = sum_n r[n,k])
    T3 = sum_{k,d} mus[k,d] * M[k,d],  M = r.T @ X

Host augments X rows to width 130: [X | 1 | xsq-slot]; xsq is filled
on-chip (ACT square + DVE per-row reduce). Per 128-row segment:
  ps[64,130] += r_seg.T @ [X | 1 | xsq]_seg
giving cols 0:128 = M, col 128 = C_k, col 129 = A_k = sum_n r[n,k]*xsq_n
(T1 = sum_k A_k). Final partial = sum([-2*mus | musq | 1] * ps).

Sharding: data-parallel over N, 16384 rows per core. Each SBUF partition
holds 128 *contiguous* rows (row order is irrelevant for every term), so
every DMA is perfectly contiguous per partition.
"""

import numpy as np

import concourse.bass as bass
import concourse.mybir as mybir
import concourse.tile as tile
from concourse import bacc
from concourse.bass_utils import run_bass_kernel_spmd

N, K, D = 131072, 64, 128
NCORES = 8
W = D + 2            # augmented row width: 128 data + ones + xsq slot
NS = N // NCORES     # rows per core
RPP = NS // 128      # rows per SBUF partition (= segments per core)
CHUNKS = 16          # DMA/compute pipeline chunks per core


def build_nc(rpp=RPP, chunks=CHUNKS):
    segs = rpp
    spc = segs // chunks       # segments per chunk
    assert spc * chunks == segs
    xf = rpp * W
    rf = rpp * K
    f32 = mybir.dt.float32

    # Bacc (not plain Bass): its compile() splits sync waits to satisfy
    # TRN2's 1-wait-per-instruction limit, which walrus enforces.
    nc = bacc.Bacc("TRN2", target_bir_lowering=False, debug=False)
    xp = nc.dram_tensor("xp", [128, xf], f32, kind="ExternalInput")
    rp = nc.dram_tensor("rp", [128, rf], f32, kind="ExternalInput")
    out = nc.dram_tensor("out", [K, W], f32, kind="ExternalOutput")

    with (
        tile.TileContext(nc) as tc,
        tc.tile_pool(name="xb", bufs=3) as xpool,
        tc.tile_pool(name="rb", bufs=3) as rpool,
        tc.tile_pool(name="scr", bufs=2) as spool,
        tc.tile_pool(name="one", bufs=1) as onepool,
        tc.tile_pool(name="ps", bufs=1, space="PSUM") as pspool,
    ):
        ps = pspool.tile([K, W], f32)

        for c in range(chunks):
            xt = xpool.tile([128, spc * W], f32, tag="x")
            rt = rpool.tile([128, spc * K], f32, tag="r")
            nc.sync.dma_start(out=xt, in_=xp[:, c * spc * W:(c + 1) * spc * W])
            nc.sync.dma_start(out=rt, in_=rp[:, c * spc * K:(c + 1) * spc * K])

            x3 = xt.rearrange("p (s w) -> p s w", w=W)
            r3 = rt.rearrange("p (s k) -> p s k", k=K)

            # per-row ||x||^2: DVE squares the chunk (table-based ACT
            # functions fault the exec unit under axon), DVE row-reduces
            # into the xsq slot (col 129 of each augmented row).
            sq = spool.tile([128, spc * D], f32, tag="sq")
            sq3 = sq.rearrange("p (s d) -> p s d", d=D)
            nc.vector.tensor_mul(sq3, x3[:, :, 0:D], x3[:, :, 0:D])
            nc.vector.reduce_sum(
                x3[:, :, D + 1:D + 2], sq3, axis=mybir.AxisListType.X
            )

            for j in range(spc):
                s = c * spc + j
                nc.tensor.matmul(
                    ps,
                    lhsT=r3[:, j, :],
                    rhs=x3[:, j, :],
                    start=(s == 0),
                    stop=(s == segs - 1),
                )

        # Ship the accumulated [K, W] panel; the final 64x130-element
        # weighted sum is part of host-side unsharding.
        osb = onepool.tile([K, W], f32)
        nc.vector.tensor_copy(osb, ps)
        nc.sync.dma_start(out=out[:, :], in_=osb)

    nc.compile()
    return nc


def make_in_maps(X, r, mus, ncores=NCORES):
    X = np.ascontiguousarray(np.asarray(X, dtype=np.float32))
    r = np.ascontiguousarray(np.asarray(r, dtype=np.float32))
    mus = np.ascontiguousarray(np.asarray(mus, dtype=np.float32))
    n = X.shape[0]
    ns = n // ncores

    in_maps = []
    for i in range(ncores):
        Xs = X[i * ns:(i + 1) * ns]
        Xa = np.empty((ns, W), np.float32)
        Xa[:, :D] = Xs
        Xa[:, D] = 1.0
        Xa[:, D + 1] = 0.0
        in_maps.append(
            {
                "xp": np.ascontiguousarray(Xa.reshape(128, (ns // 128) * W)),
                "rp": np.ascontiguousarray(
                    r[i * ns:(i + 1) * ns].reshape(128, (ns // 128) * K)
                ),
            }
        )
    return in_maps


def combine_outputs(results, mus):
    """Unshard: weighted sum of each core's [K, W] panel -> mean."""
    mus = np.asarray(mus, dtype=np.float32)
    musq = (mus.astype(np.float64) ** 2).sum(1)
    ma = np.concatenate(
        [-2.0 * mus.astype(np.float64), musq[:, None], np.ones((K, 1))], axis=1
    )
    total = 0.0
    for res in results:
        total += float((ma * res["out"].astype(np.float64)).sum())
    return np.array(total / (N * K), dtype=np.float32)


def kernel(X, r, mus):
    nc = build_nc()
    in_maps = make_in_maps(X, r, mus)
    res = run_bass_kernel_spmd(nc, in_maps, list(range(NCORES)))
    return combine_outputs(res.results[:NCORES], mus)



# revision 2
# speedup vs baseline: 2.7458x; 2.7458x over previous
"""IsoGMM loss kernel for 8 Trainium2 NeuronCores.

loss = mean_{n,k} r[n,k] * ||X[n] - mus[k]||^2

Decomposition (the entire loss folds into ONE accumulated PE matmul per core):
  sum_{n,k} r*d2 = T1 + T2 - 2*T3
    T1 = sum_n xsq_n * R_n        (xsq_n = ||X[n]||^2, R_n = sum_k r[n,k])
    T2 = sum_k musq_k * C_k       (C_k = sum_n r[n,k])
    T3 = sum_{k,d} mus[k,d] * M[k,d],  M = r.T @ X

Host augments X rows to width 132: [X | 1 | xsq*2^-4 | pad pad], all fp8
e4m3 (xsq is computed host-side from the fp32 X, so no on-chip DVE work
at all). r ships as fp8 too. Tolerance is 2e-2; measured fp8 rel err is
~7e-4 (cross/weight terms only pass through the quantized values, musq
stays fp64 on host). Per 128-row segment:
  ps[64,132] += r_seg.T @ [X | 1 | xsq]_seg       (fp8 matmul, fp32 PSUM)
giving cols 0:128 = M, col 128 = C_k, col 129 = 2^-4 * A_k
(A_k = sum_n r[n,k]*xsq_n). Final partial = sum([-2*mus | musq | 16] * ps).

Sharding: data-parallel over N, 16384 rows per core. Each SBUF partition
holds 128 *contiguous* rows (row order is irrelevant for every term), so
every DMA is perfectly contiguous per partition.
"""

import ml_dtypes
import numpy as np

import concourse.bass as bass
import concourse.mybir as mybir
import concourse.tile as tile
from concourse import bacc
from concourse.bass_utils import run_bass_kernel_spmd

N, K, D = 131072, 64, 128
NCORES = 8
W = D + 4            # augmented row width: 128 data + ones + xsq + 2 pad
NS = N // NCORES     # rows per core
RPP = NS // 128      # rows per SBUF partition (= segments per core)
CHUNKS = 4           # DMA/compute pipeline chunks per core
XSQ_SCALE = 2.0 ** -4  # keep the xsq column small in fp8 (range ~[4,14])

F8 = ml_dtypes.float8_e4m3


def build_nc(rpp=RPP, chunks=CHUNKS):
    segs = rpp
    spc = segs // chunks       # segments per chunk
    assert spc * chunks == segs
    xf = rpp * W
    rf = rpp * K
    f32 = mybir.dt.float32
    f8 = mybir.dt.float8e4

    # Bacc (not plain Bass): its compile() splits sync waits to satisfy
    # TRN2's 1-wait-per-instruction limit, which walrus enforces.
    nc = bacc.Bacc("TRN2", target_bir_lowering=False, debug=False)
    xp = nc.dram_tensor("xp", [128, xf], f8, kind="ExternalInput")
    rp = nc.dram_tensor("rp", [128, rf], f8, kind="ExternalInput")
    out = nc.dram_tensor("out", [K, W], f32, kind="ExternalOutput")

    with (
        tile.TileContext(nc) as tc,
        tc.tile_pool(name="xb", bufs=chunks) as xpool,
        tc.tile_pool(name="rb", bufs=chunks) as rpool,
        tc.tile_pool(name="one", bufs=1) as onepool,
        tc.tile_pool(name="ps", bufs=1, space="PSUM") as pspool,
    ):
        ps = pspool.tile([K, W], f32)

        # Issue every chunk's DMA up front: bufs=chunks means no pool
        # recycling, so all 16 DMA engines stream from t=0 while the PE
        # chews through chunks in order.
        xts, rts = [], []
        for c in range(chunks):
            xt = xpool.tile([128, spc * W], f8, tag="x")
            rt = rpool.tile([128, spc * K], f8, tag="r")
            nc.sync.dma_start(out=xt, in_=xp[:, c * spc * W:(c + 1) * spc * W])
            nc.sync.dma_start(out=rt, in_=rp[:, c * spc * K:(c + 1) * spc * K])
            xts.append(xt)
            rts.append(rt)

        for c in range(chunks):
            x3 = xts[c].rearrange("p (s w) -> p s w", w=W)
            r3 = rts[c].rearrange("p (s k) -> p s k", k=K)
            for j in range(spc):
                s = c * spc + j
                nc.tensor.matmul(
                    ps,
                    lhsT=r3[:, j, :],
                    rhs=x3[:, j, :],
                    start=(s == 0),
                    stop=(s == segs - 1),
                )

        # Ship the accumulated [K, W] panel; the final 64x132-element
        # weighted sum is part of host-side unsharding.
        osb = onepool.tile([K, W], f32)
        nc.vector.tensor_copy(osb, ps)
        nc.sync.dma_start(out=out[:, :], in_=osb)

    nc.compile()
    return nc


def make_in_maps(X, r, mus, ncores=NCORES):
    X = np.ascontiguousarray(np.asarray(X, dtype=np.float32))
    r = np.ascontiguousarray(np.asarray(r, dtype=np.float32))
    n = X.shape[0]
    ns = n // ncores

    # Host-side row norms from the full-precision X (the only biased term
    # if it were computed from quantized X), then quantize everything.
    xsq = np.einsum("nd,nd->n", X, X, dtype=np.float32)
    Xa = np.zeros((n, W), F8)
    Xa[:, :D] = X.astype(F8)
    Xa[:, D] = F8(1.0)
    Xa[:, D + 1] = (xsq * XSQ_SCALE).astype(F8)
    r8 = r.astype(F8)

    in_maps = []
    for i in range(ncores):
        in_maps.append(
            {
                "xp": np.ascontiguousarray(
                    Xa[i * ns:(i + 1) * ns].reshape(128, (ns // 128) * W)
                ),
                "rp": np.ascontiguousarray(
                    r8[i * ns:(i + 1) * ns].reshape(128, (ns // 128) * K)
                ),
            }
        )
    return in_maps


def combine_outputs(results, mus):
    """Unshard: weighted sum of each core's [K, W] panel -> mean."""
    mus = np.asarray(mus, dtype=np.float32)
    musq = (mus.astype(np.float64) ** 2).sum(1)
    ma = np.concatenate(
        [
            -2.0 * mus.astype(np.float64),
            musq[:, None],
            np.full((K, 1), 1.0 / XSQ_SCALE),
            np.zeros((K, 2)),
        ],
        axis=1,
    )
    total = 0.0
    for res in results:
        total += float((ma * res["out"].astype(np.float64)).sum())
    return np.array(total / (N * K), dtype=np.float32)


def kernel(X, r, mus):
    nc = build_nc()
    in_maps = make_in_maps(X, r, mus)
    res = run_bass_kernel_spmd(nc, in_maps, list(range(NCORES)))
    return combine_outputs(res.results[:NCORES], mus)


# revision 3
# speedup vs baseline: 2.7525x; 1.0025x over previous
"""IsoGMM loss kernel for 8 Trainium2 NeuronCores.

loss = mean_{n,k} r[n,k] * ||X[n] - mus[k]||^2

Decomposition (the entire loss folds into ONE accumulated PE matmul per core):
  sum_{n,k} r*d2 = T1 + T2 - 2*T3
    T1 = sum_n xsq_n * R_n        (xsq_n = ||X[n]||^2, R_n = sum_k r[n,k])
    T2 = sum_k musq_k * C_k       (C_k = sum_n r[n,k])
    T3 = sum_{k,d} mus[k,d] * M[k,d],  M = r.T @ X

Host augments X rows to width 132: [X | 1 | xsq*2^-4 | pad pad], all fp8
e4m3 (xsq is computed host-side from the fp32 X, so no on-chip DVE work
at all). r ships as fp8 too. Tolerance is 2e-2; measured fp8 rel err is
~7e-4 (cross/weight terms only pass through the quantized values, musq
stays fp64 on host). Per 128-row segment:
  ps[64,132] += r_seg.T @ [X | 1 | xsq]_seg       (fp8 matmul, fp32 PSUM)
giving cols 0:128 = M, col 128 = C_k, col 129 = 2^-4 * A_k
(A_k = sum_n r[n,k]*xsq_n). Final partial = sum([-2*mus | musq | 16] * ps).

Perf notes (from NTFF traces):
- Each DMA_DIRECT2D doorbell costs ~630 ns *serialized* on its issuing
  queue, so X and r are packed into ONE dram tensor with chunk-major
  layout -> one doorbell per chunk, alternating the sync and scalar
  (Activation) queues (both are hwdge engines) to overlap issue.
- Chunk sizes are front/back-loaded small: the first chunk lands early
  so the PE starts sooner; the last is small so the PE tail after the
  final byte is short.
- fp8e4 DoubleRow matmuls contract two 128-row segments per instruction
  (0.5 cycles/row), halving both PE streaming time and LDWEIGHTS count.

Sharding: data-parallel over N, 16384 rows per core. Each SBUF partition
holds 128 *contiguous* rows (row order is irrelevant for every term), so
every DMA is perfectly contiguous per partition.
"""

import ml_dtypes
import numpy as np

import concourse.bass as bass
import concourse.mybir as mybir
import concourse.tile as tile
from concourse import bacc
from concourse.bass_utils import run_bass_kernel_spmd

N, K, D = 131072, 64, 128
NCORES = 8
W = D + 4            # augmented row width: 128 data + ones + xsq + 2 pad
NS = N // NCORES     # rows per core
RPP = NS // 128      # rows per SBUF partition (= segments per core)
CHUNK_SEGS = (16, 32, 32, 32, 16)   # segments per pipeline chunk
XSQ_SCALE = 2.0 ** -4  # keep the xsq column small in fp8 (range ~[4,14])
DOUBLE_ROW = True

F8 = ml_dtypes.float8_e4m3
BPS = W + K          # bytes per row-segment slot in the packed layout


def build_nc(chunk_segs=CHUNK_SEGS, double_row=DOUBLE_ROW):
    segs = RPP
    assert sum(chunk_segs) == segs
    f32 = mybir.dt.float32
    f8 = mybir.dt.float8e4

    # Bacc (not plain Bass): its compile() splits sync waits to satisfy
    # TRN2's 1-wait-per-instruction limit, which walrus enforces.
    nc = bacc.Bacc("TRN2", target_bir_lowering=False, debug=False)
    xr = nc.dram_tensor("xr", [128, segs * BPS], f8, kind="ExternalInput")
    out = nc.dram_tensor("out", [K, W], f32, kind="ExternalOutput")

    with (
        tile.TileContext(nc) as tc,
        tc.tile_pool(name="xb", bufs=len(chunk_segs)) as xpool,
        tc.tile_pool(name="one", bufs=1) as onepool,
        tc.tile_pool(name="ps", bufs=1, space="PSUM") as pspool,
    ):
        ps = pspool.tile([K, W], f32)

        # One doorbell per chunk, alternating issue queues; all issued up
        # front (bufs == n_chunks, no pool recycling) so the 16 DMA
        # engines stream continuously from t=0.
        tiles = []
        off = 0
        for c, spc in enumerate(chunk_segs):
            t = xpool.tile([128, spc * BPS], f8, tag="xr")
            eng = nc.sync if c % 2 == 0 else nc.scalar
            eng.dma_start(out=t, in_=xr[:, off:off + spc * BPS])
            tiles.append(t)
            off += spc * BPS

        s = 0
        for c, spc in enumerate(chunk_segs):
            t = tiles[c]
            x3 = t[:, :spc * W].rearrange("p (s w) -> p s w", w=W)
            r3 = t[:, spc * W:].rearrange("p (s k) -> p s k", k=K)
            if double_row:
                for j in range(spc // 2):
                    nc.tensor.matmul(
                        ps,
                        lhsT=r3[:, 2 * j:2 * j + 2, :],
                        rhs=x3[:, 2 * j:2 * j + 2, :],
                        start=(s == 0),
                        stop=(s == segs - 2),
                        perf_mode=mybir.MatmulPerfMode.DoubleRow,
                    )
                    s += 2
            else:
                for j in range(spc):
                    nc.tensor.matmul(
                        ps,
                        lhsT=r3[:, j, :],
                        rhs=x3[:, j, :],
                        start=(s == 0),
                        stop=(s == segs - 1),
                    )
                    s += 1

        # Ship the accumulated [K, W] panel; the final 64x132-element
        # weighted sum is part of host-side unsharding.
        osb = onepool.tile([K, W], f32)
        nc.vector.tensor_copy(osb, ps)
        nc.scalar.dma_start(out=out[:, :], in_=osb)

    nc.compile()
    return nc


def make_in_maps(X, r, mus, ncores=NCORES, chunk_segs=CHUNK_SEGS):
    X = np.ascontiguousarray(np.asarray(X, dtype=np.float32))
    r = np.ascontiguousarray(np.asarray(r, dtype=np.float32))
    n = X.shape[0]
    ns = n // ncores

    # Host-side row norms from the full-precision X (the only biased term
    # if it were computed from quantized X), then quantize everything.
    xsq = np.einsum("nd,nd->n", X, X, dtype=np.float32)
    Xa = np.zeros((n, W), F8)
    Xa[:, :D] = X.astype(F8)
    Xa[:, D] = F8(1.0)
    Xa[:, D + 1] = (xsq * XSQ_SCALE).astype(F8)
    r8 = r.astype(F8)

    in_maps = []
    for i in range(ncores):
        x4 = Xa[i * ns:(i + 1) * ns].reshape(128, RPP, W)
        r4 = r8[i * ns:(i + 1) * ns].reshape(128, RPP, K)
        blocks = []
        s = 0
        for spc in chunk_segs:
            blocks.append(x4[:, s:s + spc].reshape(128, spc * W))
            blocks.append(r4[:, s:s + spc].reshape(128, spc * K))
            s += spc
        in_maps.append({"xr": np.ascontiguousarray(np.concatenate(blocks, axis=1))})
    return in_maps


def combine_outputs(results, mus):
    """Unshard: weighted sum of each core's [K, W] panel -> mean."""
    mus = np.asarray(mus, dtype=np.float32)
    musq = (mus.astype(np.float64) ** 2).sum(1)
    ma = np.concatenate(
        [
            -2.0 * mus.astype(np.float64),
            musq[:, None],
            np.full((K, 1), 1.0 / XSQ_SCALE),
            np.zeros((K, 2)),
        ],
        axis=1,
    )
    total = 0.0
    for res in results:
        total += float((ma * res["out"].astype(np.float64)).sum())
    return np.array(total / (N * K), dtype=np.float32)


def kernel(X, r, mus):
    nc = build_nc()
    in_maps = make_in_maps(X, r, mus)
    res = run_bass_kernel_spmd(nc, in_maps, list(range(NCORES)))
    return combine_outputs(res.results[:NCORES], mus)


# revision 8
# speedup vs baseline: 3.2546x; 1.1824x over previous
"""IsoGMM loss kernel for 8 Trainium2 NeuronCores.

loss = mean_{n,k} r[n,k] * ||X[n] - mus[k]||^2

Decomposition (the entire loss folds into ONE accumulated PE matmul per core):
  sum_{n,k} r*d2 = T1 + T2 - 2*T3
    T1 = sum_n xsq_n * R_n        (xsq_n = ||X[n]||^2, R_n = sum_k r[n,k])
    T2 = sum_k musq_k * C_k       (C_k = sum_n r[n,k])
    T3 = sum_{k,d} mus[k,d] * M[k,d],  M = r.T @ X

Host augments X rows to width 132: [X | 1 | xsq*2^-4 | pad pad], all fp8
e4m3 (xsq is computed host-side from the fp32 X, so no on-chip DVE work
at all). r ships as fp8 too. Tolerance is 2e-2; measured fp8 rel err is
~7e-4 (cross/weight terms only pass through the quantized values, musq
stays fp64 on host). Per 128-row segment:
  ps[64,132] += r_seg.T @ [X | 1 | xsq]_seg       (fp8 matmul, fp32 PSUM)
giving cols 0:128 = M, col 128 = C_k, col 129 = 2^-4 * A_k
(A_k = sum_n r[n,k]*xsq_n). Final partial = sum([-2*mus | musq | 16] * ps).

Perf notes (from NTFF traces):
- Each DMA_DIRECT2D doorbell costs ~700 ns *serialized* on its issuing
  queue, so X and r are packed into ONE dram tensor with chunk-major
  layout -> one doorbell per chunk. All input doorbells go on the sync
  queue IN CONSUMPTION ORDER: with two issuing queues the 16 DMA engines
  round-robin between the queue streams and chunk completions arrive
  out of order, starving the PE mid-kernel.
- The DMA engines have a ~1.2 us cold-start on their first packets; a
  tiny warmup DMA issued on the (otherwise idle) scalar queue right at
  kernel start absorbs it before chunk 0's real doorbell rings.
- The first chunk is small so the PE starts early; after that the PE
  (~135 ns per DoubleRow pair = 67.5 ns/segment) and the DMA stream
  (~67 ns/segment) are rate-matched, so chunking is just granularity.
- fp8e4 DoubleRow matmuls contract two 128-row segments per instruction
  (0.5 cycles/row), halving PE instruction count (the per-instruction
  issue cost, not the stream time, is what bounds the PE here).

Sharding: data-parallel over N, 16384 rows per core. Each SBUF partition
holds 128 *contiguous* rows (row order is irrelevant for every term), so
every DMA is perfectly contiguous per partition.
"""

import ml_dtypes
import numpy as np

import concourse.bass as bass
import concourse.mybir as mybir
import concourse.tile as tile
from concourse import bacc
from concourse.bass_utils import run_bass_kernel_spmd

N, K, D = 131072, 64, 128
NCORES = 8
W = D + 4            # augmented row width: 128 data + ones + xsq + 2 pad
NS = N // NCORES     # rows per core
RPP = NS // 128      # rows per SBUF partition (= segments per core)
CHUNK_SEGS = (8, 24, 24, 24, 24, 24)   # segments per pipeline chunk
XSQ_SCALE = 2.0 ** -4  # keep the xsq column small in fp8 (range ~[4,14])
DOUBLE_ROW = True

F8 = ml_dtypes.float8_e4m3
BPS = W + K          # bytes per row-segment slot in the packed layout


def build_nc(chunk_segs=CHUNK_SEGS, double_row=DOUBLE_ROW):
    segs = RPP
    assert sum(chunk_segs) == segs
    f32 = mybir.dt.float32
    f8 = mybir.dt.float8e4

    # Bacc (not plain Bass): its compile() splits sync waits to satisfy
    # TRN2's 1-wait-per-instruction limit, which walrus enforces.
    nc = bacc.Bacc("TRN2", target_bir_lowering=False, debug=False)
    xr = nc.dram_tensor("xr", [128, segs * BPS], f8, kind="ExternalInput")
    out = nc.dram_tensor("out", [K, W], f32, kind="ExternalOutput")

    with (
        tile.TileContext(nc) as tc,
        tc.tile_pool(name="xb", bufs=len(chunk_segs)) as xpool,
        tc.tile_pool(name="one", bufs=1) as onepool,
        tc.tile_pool(name="wrm", bufs=1) as warmpool,
        tc.tile_pool(name="ps", bufs=1, space="PSUM") as pspool,
    ):
        ps = pspool.tile([K, W], f32)

        # Warmup: a 4-byte-per-line read issued on the scalar queue rings
        # the DMA engines immediately so their cold-start happens before
        # chunk 0's doorbell (the result is never read).
        warm = warmpool.tile([128, 4], f8, tag="warm")
        nc.scalar.dma_start(out=warm, in_=xr[:, 0:4])

        # One doorbell per chunk, all on the sync queue in consumption
        # order; all issued up front (bufs == n_chunks, no pool
        # recycling) so the 16 DMA engines stream continuously.
        tiles = []
        off = 0
        for c, spc in enumerate(chunk_segs):
            t = xpool.tile([128, spc * BPS], f8, tag="xr")
            nc.sync.dma_start(out=t, in_=xr[:, off:off + spc * BPS])
            tiles.append(t)
            off += spc * BPS

        s = 0
        for c, spc in enumerate(chunk_segs):
            t = tiles[c]
            x3 = t[:, :spc * W].rearrange("p (s w) -> p s w", w=W)
            r3 = t[:, spc * W:].rearrange("p (s k) -> p s k", k=K)
            if double_row:
                for j in range(spc // 2):
                    nc.tensor.matmul(
                        ps,
                        lhsT=r3[:, 2 * j:2 * j + 2, :],
                        rhs=x3[:, 2 * j:2 * j + 2, :],
                        start=(s == 0),
                        stop=(s == segs - 2),
                        perf_mode=mybir.MatmulPerfMode.DoubleRow,
                    )
                    s += 2
            else:
                for j in range(spc):
                    nc.tensor.matmul(
                        ps,
                        lhsT=r3[:, j, :],
                        rhs=x3[:, j, :],
                        start=(s == 0),
                        stop=(s == segs - 1),
                    )
                    s += 1

        # Ship the accumulated [K, W] panel; the final 64x132-element
        # weighted sum is part of host-side unsharding.
        osb = onepool.tile([K, W], f32)
        nc.vector.tensor_copy(osb, ps)
        nc.scalar.dma_start(out=out[:, :], in_=osb)

    nc.compile()
    return nc


def make_in_maps(X, r, mus, ncores=NCORES, chunk_segs=CHUNK_SEGS):
    X = np.ascontiguousarray(np.asarray(X, dtype=np.float32))
    r = np.ascontiguousarray(np.asarray(r, dtype=np.float32))
    n = X.shape[0]
    ns = n // ncores

    # Host-side row norms from the full-precision X (the only biased term
    # if it were computed from quantized X), then quantize everything.
    xsq = np.einsum("nd,nd->n", X, X, dtype=np.float32)
    Xa = np.zeros((n, W), F8)
    Xa[:, :D] = X.astype(F8)
    Xa[:, D] = F8(1.0)
    Xa[:, D + 1] = (xsq * XSQ_SCALE).astype(F8)
    r8 = r.astype(F8)

    in_maps = []
    for i in range(ncores):
        x4 = Xa[i * ns:(i + 1) * ns].reshape(128, RPP, W)
        r4 = r8[i * ns:(i + 1) * ns].reshape(128, RPP, K)
        blocks = []
        s = 0
        for spc in chunk_segs:
            blocks.append(x4[:, s:s + spc].reshape(128, spc * W))
            blocks.append(r4[:, s:s + spc].reshape(128, spc * K))
            s += spc
        in_maps.append({"xr": np.ascontiguousarray(np.concatenate(blocks, axis=1))})
    return in_maps


def combine_outputs(results, mus):
    """Unshard: weighted sum of each core's [K, W] panel -> mean."""
    mus = np.asarray(mus, dtype=np.float32)
    musq = (mus.astype(np.float64) ** 2).sum(1)
    ma = np.concatenate(
        [
            -2.0 * mus.astype(np.float64),
            musq[:, None],
            np.full((K, 1), 1.0 / XSQ_SCALE),
            np.zeros((K, 2)),
        ],
        axis=1,
    )
    total = 0.0
    for res in results:
        total += float((ma * res["out"].astype(np.float64)).sum())
    return np.array(total / (N * K), dtype=np.float32)


def kernel(X, r, mus):
    nc = build_nc()
    in_maps = make_in_maps(X, r, mus)
    res = run_bass_kernel_spmd(nc, in_maps, list(range(NCORES)))
    return combine_outputs(res.results[:NCORES], mus)
